# revision 2
# baseline (speedup 1.0000x reference)
"""Trainium2 Bass kernel for nn_Contrast_Item (contrastive loss over 8192x8192 sim matrix).

Strategy (8-way row sharding, SPMD, rank-agnostic program):
  Phase 1: each core projects its 1024-row shard of z_mp/z_sc through the
           Linear->ELU->Linear MLP in *transposed* activation layout
           (zT -> HT -> PT, hidden on partitions), L2-normalizes
           (inv_norm = exp(-0.5*ln(sum(P^2))) -- avoids the low-precision
           Sqrt table and DVE reciprocal), then AllGathers the normalized
           bf16 projections so every core holds [128, 8192] for both tensors.
  Phase 2: per core, two 1024x8192 exp-similarity blocks:
           A = exp(nmp_i . nsc_j / T) for its mp rows (mp2sc numerators),
           B = exp(nsc_i . nmp_j / T) for its sc rows (sc2mp via sim.T rows).
           Row sums come free from ACT's accum_out during the exp pass;
           pos-masked row sums via DVE scalar_tensor_tensor (A * pos, accum).
           pos is DMA-cast int32->bf16 on the SWDGE path during load.
  Final:   loss_i = -(LAM*log(mdA/rsA + eps) + (1-LAM)*log(mdB/rsB + eps)).

kernel(**inputs) takes FULL inputs, shards internally, returns FULL [8192] output.
"""

import os
import sys
from functools import lru_cache

sys.path.insert(0, "/opt/trn_rl_repo")

import numpy as np

import concourse.bass as bass
import concourse.bacc as bacc
import concourse.tile as tile
import concourse.mybir as mybir
import concourse.masks as masks
from concourse.bass_utils import run_bass_kernel_spmd

F32 = mybir.dt.float32
BF16 = mybir.dt.bfloat16
I32 = mybir.dt.int32
AF = mybir.ActivationFunctionType
OP = mybir.AluOpType

N = 8192          # rows total
NSH = N // 8      # rows per core = 1024
C = 256           # input feature dim
H = 128           # hidden dim
TEMP = 0.2
LAM = 0.5
EPS = 1e-8

RC = NSH // 128   # row chunks per core = 8
CG = 2048         # column-group width for phase-2 tiles
NCG = N // CG     # = 4
N_CORES = 8

# pos handling: "dma_cast" (SWDGE int32->bf16 during DMA), "gpsimd_copy", "int32_direct"
POS_MODE = os.environ.get("POS_MODE", "dma_cast")
# masked-sum engine split: fraction of (rc,cg,b) iterations handled by gpsimd
GPSIMD_FRAC = float(os.environ.get("GPSIMD_FRAC", "0.25"))


def _proj_transposed(nc, tc, pools, z_dram, w1, b1sb, w2, b2sb, ident, ones, out_bf16):
    """Project one z shard [NSH, C] -> normalized transposed bf16 [128, NSH]."""
    zrow_pool, tp_psum, zt_pool, proj_psum, mlp_pool, norm_psum = pools

    # transpose z shard into zT (c-major): zT[c][128, NSH] for c-chunk 0/1
    zt = [zt_pool.tile([128, NSH], F32, tag=f"zt{c}", name=f"zt{c}") for c in range(2)]
    for r8 in range(RC):
        zrow = zrow_pool.tile([128, C], F32, tag="zrow")
        nc.sync.dma_start(zrow[:], z_dram[128 * r8:128 * (r8 + 1), :])
        for c in range(2):
            tp = tp_psum.tile([128, 128], F32, tag="tp")
            nc.tensor.transpose(tp[:], zrow[:, 128 * c:128 * (c + 1)], ident[:])
            nc.any.tensor_copy(zt[c][:, 128 * r8:128 * (r8 + 1)], tp[:])

    # L1: HT = W1.T @ zT  (K=256 via 2 chunks), +b1, ELU
    hp = proj_psum.tile([128, NSH], F32, tag="proj")
    for nh in range(NSH // 512):
        for c in range(2):
            nc.tensor.matmul(hp[:, 512 * nh:512 * (nh + 1)], w1[c][:],
                             zt[c][:, 512 * nh:512 * (nh + 1)],
                             start=(c == 0), stop=(c == 1))
    # elu(x) = max(x, min(exp(x)-1, 0)), x = hp + b1
    e = mlp_pool.tile([128, NSH], F32, tag="e")
    nc.scalar.activation(e[:], hp[:], AF.Exp, bias=b1sb[:])
    tm = mlp_pool.tile([128, NSH], F32, tag="tm")
    nc.vector.tensor_scalar(tm[:], e[:], 1.0, 0.0, OP.subtract, OP.min)
    hh = mlp_pool.tile([128, NSH], F32, tag="hh")
    nc.vector.scalar_tensor_tensor(hh[:], hp[:], b1sb[:], tm[:], OP.add, OP.max)

    # L2: PT = W2.T @ H
    pp = proj_psum.tile([128, NSH], F32, tag="proj")
    for nh in range(NSH // 512):
        nc.tensor.matmul(pp[:, 512 * nh:512 * (nh + 1)], w2[:],
                         hh[:, 512 * nh:512 * (nh + 1)], start=True, stop=True)

    # P+b2 and (P+b2)^2; norms via ones-matmul (partition reduce)
    pb = mlp_pool.tile([128, NSH], F32, tag="pb")
    nc.scalar.activation(pb[:], pp[:], AF.Identity, bias=b2sb[:])
    sq = mlp_pool.tile([128, NSH], F32, tag="sq")
    nc.scalar.activation(sq[:], pp[:], AF.Square, bias=b2sb[:])
    nq = norm_psum.tile([1, NSH], F32, tag="nq")
    for nh in range(NSH // 512):
        nc.tensor.matmul(nq[:, 512 * nh:512 * (nh + 1)], ones[:],
                         sq[:, 512 * nh:512 * (nh + 1)], start=True, stop=True)
    # inv_norm = nq^-0.5 = exp(-0.5 * ln(nq))
    lnq = mlp_pool.tile([1, NSH], F32, tag="lnq")
    nc.scalar.activation(lnq[:], nq[:], AF.Ln)
    inv = mlp_pool.tile([1, NSH], F32, tag="inv")
    nc.scalar.activation(inv[:], lnq[:], AF.Exp, scale=-0.5)
    invb = mlp_pool.tile([128, NSH], F32, tag="invb")
    nc.gpsimd.partition_broadcast(invb[:], inv[:])
    nc.vector.tensor_tensor(out_bf16[:], pb[:], invb[:], OP.mult)


@lru_cache(maxsize=1)
def build_kernel():
    nc = bacc.Bacc("TRN2", target_bir_lowering=False, debug=False,
                   num_devices=N_CORES)

    z_mp_sh = nc.dram_tensor("z_mp_sh", [NSH, C], F32, kind="ExternalInput")
    z_sc_sh = nc.dram_tensor("z_sc_sh", [NSH, C], F32, kind="ExternalInput")
    w1_d = nc.dram_tensor("W1", [C, H], F32, kind="ExternalInput")
    b1_d = nc.dram_tensor("b1", [H, 1], F32, kind="ExternalInput")
    w2_d = nc.dram_tensor("W2", [H, H], F32, kind="ExternalInput")
    b2_d = nc.dram_tensor("b2", [H, 1], F32, kind="ExternalInput")
    pos_d = nc.dram_tensor("pos_sh", [NSH, N], I32, kind="ExternalInput")
    loss_d = nc.dram_tensor("loss_sh", [NSH], F32, kind="ExternalOutput")

    with tile.TileContext(nc) as tc:
        from contextlib import ExitStack
        with tc.tile_pool(name="const", bufs=1) as const_pool, \
             tc.tile_pool(name="persist", bufs=1) as persist_pool, \
             tc.tile_pool(name="dram", bufs=1, space="DRAM") as dram_pool:

            ident = const_pool.tile([128, 128], F32)
            masks.make_identity(nc, ident[:])
            ones = const_pool.tile([128, 1], F32)
            nc.gpsimd.memset(ones[:], 1.0)
            w1 = [const_pool.tile([128, H], F32, tag=f"w1_{c}", name=f"w1_{c}") for c in range(2)]
            for c in range(2):
                nc.sync.dma_start(w1[c][:], w1_d[128 * c:128 * (c + 1), :])
            w2 = const_pool.tile([128, H], F32)
            nc.sync.dma_start(w2[:], w2_d[:, :])
            b1sb = const_pool.tile([128, 1], F32)
            nc.sync.dma_start(b1sb[:], b1_d[:, :])
            b2sb = const_pool.tile([128, 1], F32)
            nc.sync.dma_start(b2sb[:], b2_d[:, :])

            # local normalized transposed shards (bf16) - also matmul stationaries
            loc = [persist_pool.tile([128, NSH], BF16, tag=f"loc{t}", name=f"loc{t}") for t in range(2)]
            # gathered full tensors
            full = [persist_pool.tile([128, N], BF16, tag=f"full{t}", name=f"full{t}") for t in range(2)]

            # ---------------- Phase 1 ----------------
            with ExitStack() as p1:
                zrow_pool = p1.enter_context(tc.tile_pool(name="zrow", bufs=2))
                tp_psum = p1.enter_context(tc.tile_pool(name="tp", bufs=2, space="PSUM"))
                zt_pool = p1.enter_context(tc.tile_pool(name="zt", bufs=1))
                proj_psum = p1.enter_context(tc.tile_pool(name="pj", bufs=2, space="PSUM"))
                mlp_pool = p1.enter_context(tc.tile_pool(name="mlp", bufs=2))
                norm_psum = p1.enter_context(tc.tile_pool(name="nq", bufs=1, space="PSUM"))
                pools = (zrow_pool, tp_psum, zt_pool, proj_psum, mlp_pool, norm_psum)

                _proj_transposed(nc, tc, pools, z_mp_sh, w1, b1sb, w2, b2sb,
                                 ident, ones, loc[0])
                _proj_transposed(nc, tc, pools, z_sc_sh, w1, b1sb, w2, b2sb,
                                 ident, ones, loc[1])

                cc_in = dram_pool.tile([128, 2 * NSH], BF16)
                cc_out = dram_pool.tile([N_CORES * 128, 2 * NSH], BF16,
                                        addr_space="Shared")
                for t in range(2):
                    nc.sync.dma_start(cc_in[:, NSH * t:NSH * (t + 1)], loc[t][:])
                nc.gpsimd.collective_compute(
                    "AllGather", OP.bypass,
                    replica_groups=[list(range(N_CORES))],
                    ins=[cc_in.opt()], outs=[cc_out.opt()],
                )
                for k in range(N_CORES):
                    for t in range(2):
                        nc.sync.dma_start(
                            full[t][:, NSH * k:NSH * (k + 1)],
                            cc_out[128 * k:128 * (k + 1), NSH * t:NSH * (t + 1)])

            # ---------------- Phase 2 ----------------
            with ExitStack() as p2:
                pos_pool = p2.enter_context(tc.tile_pool(name="pos", bufs=4))
                s_psum = p2.enter_context(tc.tile_pool(name="s", bufs=2, space="PSUM"))
                a_pool = p2.enter_context(tc.tile_pool(name="a", bufs=3))
                scr_pool = p2.enter_context(tc.tile_pool(name="scr", bufs=2))
                acc_pool = p2.enter_context(tc.tile_pool(name="acc", bufs=1))

                rs_all = [acc_pool.tile([128, RC * NCG], F32, tag=f"rs{b}", name=f"rs{b}") for b in range(2)]
                md_all = [acc_pool.tile([128, RC * NCG], F32, tag=f"md{b}", name=f"md{b}") for b in range(2)]

                n_iter = RC * NCG * 2
                n_gps = int(n_iter * GPSIMD_FRAC)
                it = 0
                for rc in range(RC):
                    for cg in range(NCG):
                        if POS_MODE == "int32_direct":
                            pos_t = pos_pool.tile([128, CG], I32, tag="pos")
                            nc.sync.dma_start(
                                pos_t[:], pos_d[128 * rc:128 * (rc + 1), CG * cg:CG * (cg + 1)])
                        elif POS_MODE == "gpsimd_copy":
                            pos_i = pos_pool.tile([128, CG], I32, tag="posi")
                            nc.sync.dma_start(
                                pos_i[:], pos_d[128 * rc:128 * (rc + 1), CG * cg:CG * (cg + 1)])
                            pos_t = pos_pool.tile([128, CG], BF16, tag="pos")
                            nc.gpsimd.tensor_copy(pos_t[:], pos_i[:])
                        else:  # dma_cast
                            pos_t = pos_pool.tile([128, CG], BF16, tag="pos")
                            nc.gpsimd.dma_start(
                                pos_t[:], pos_d[128 * rc:128 * (rc + 1), CG * cg:CG * (cg + 1)])
                        for b in range(2):
                            S = s_psum.tile([128, CG], F32, tag="S")
                            lhsT = loc[b][:, 128 * rc:128 * (rc + 1)]
                            rhs = full[1 - b]
                            for c4 in range(CG // 512):
                                nc.tensor.matmul(
                                    S[:, 512 * c4:512 * (c4 + 1)], lhsT,
                                    rhs[:, CG * cg + 512 * c4:CG * cg + 512 * (c4 + 1)],
                                    start=True, stop=True)
                            col = NCG * rc + cg
                            a_t = a_pool.tile([128, CG], BF16, tag="a")
                            nc.scalar.activation(a_t[:], S[:], AF.Exp, scale=1.0 / TEMP,
                                                 accum_out=rs_all[b][:, col:col + 1])
                            scr = scr_pool.tile([128, CG], BF16, tag="scr")
                            eng = nc.gpsimd if it < n_gps else nc.vector
                            eng.scalar_tensor_tensor(
                                scr[:], a_t[:], 1.0, pos_t[:], OP.bypass, OP.mult,
                                accum_out=md_all[b][:, col:col + 1])
                            it += 1

                # ---------------- Final loss ----------------
                fin = acc_pool.tile([128, RC], F32, tag="fin_dummy")  # noqa (keeps pool)
                ls = []
                for b in range(2):
                    rs_red = acc_pool.tile([128, RC], F32, tag=f"rsr{b}")
                    nc.vector.tensor_reduce(
                        rs_red[:], rs_all[b][:].rearrange("p (r c) -> p r c", c=NCG),
                        mybir.AxisListType.X, OP.add)
                    md_red = acc_pool.tile([128, RC], F32, tag=f"mdr{b}")
                    nc.vector.tensor_reduce(
                        md_red[:], md_all[b][:].rearrange("p (r c) -> p r c", c=NCG),
                        mybir.AxisListType.X, OP.add)
                    rse = acc_pool.tile([128, RC], F32, tag=f"rse{b}")
                    nc.vector.tensor_scalar(rse[:], rs_red[:], EPS, None, OP.add)
                    rcp = acc_pool.tile([128, RC], F32, tag=f"rcp{b}")
                    nc.vector.reciprocal(rcp[:], rse[:])
                    q = acc_pool.tile([128, RC], F32, tag=f"q{b}")
                    nc.vector.tensor_tensor(q[:], md_red[:], rcp[:], OP.mult)
                    qe = acc_pool.tile([128, RC], F32, tag=f"qe{b}")
                    nc.vector.tensor_scalar(qe[:], q[:], EPS, None, OP.add)
                    lb = acc_pool.tile([128, RC], F32, tag=f"l{b}")
                    nc.scalar.activation(lb[:], qe[:], AF.Ln)
                    ls.append(lb)
                l1s = acc_pool.tile([128, RC], F32, tag="l1s")
                nc.vector.tensor_scalar(l1s[:], ls[1][:], -(1.0 - LAM), None, OP.mult)
                ans = acc_pool.tile([128, RC], F32, tag="ans")
                nc.vector.scalar_tensor_tensor(ans[:], ls[0][:], -LAM, l1s[:],
                                               OP.mult, OP.add)
                nc.sync.dma_start(loss_d.ap().rearrange("(r p) -> p r", p=128), ans[:])

    nc.compile()
    return nc


def kernel(**inputs):
    nc = build_kernel()
    z_mp = np.ascontiguousarray(inputs["z_mp"], dtype=np.float32)
    z_sc = np.ascontiguousarray(inputs["z_sc"], dtype=np.float32)
    W1 = np.ascontiguousarray(inputs["W1"], dtype=np.float32)
    b1 = np.ascontiguousarray(inputs["b1"], dtype=np.float32).reshape(H, 1)
    W2 = np.ascontiguousarray(inputs["W2"], dtype=np.float32)
    b2 = np.ascontiguousarray(inputs["b2"], dtype=np.float32).reshape(H, 1)
    pos = np.ascontiguousarray(inputs["pos"], dtype=np.int32)

    in_maps = []
    for k in range(N_CORES):
        sl = slice(NSH * k, NSH * (k + 1))
        in_maps.append({
            "z_mp_sh": z_mp[sl], "z_sc_sh": z_sc[sl],
            "W1": W1, "b1": b1, "W2": W2, "b2": b2,
            "pos_sh": np.ascontiguousarray(pos[sl]),
        })
    res = run_bass_kernel_spmd(nc, in_maps, core_ids=list(range(N_CORES)),
                               trace=bool(int(os.environ.get("KERNEL_TRACE", "0"))))
    out = np.concatenate([res.results[k]["loss_sh"] for k in range(N_CORES)])
    kernel.last_results = res
    return out.astype(np.float32)
